# revision 1
# baseline (speedup 1.0000x reference)
"""AmpNorm Trainium2 kernel: FFT-domain amplitude normalization.

reference semantics:
    fft = fft2(x); amp = fftshift(|fft|); pha = angle(fft)
    amp_mean = mean(amp, axis=0)
    new_amp = (1-m)*running_amp + m*amp_mean     (EMA branch; init branch if sum==0)
    out = real(ifft2(ifftshift(new_amp) * exp(i*pha)))

Device formulation (per [512,512] image; shifts absorbed on host):
    Z = F @ X @ F, computed only for column frequencies k_c in [0, 256]
    (x real => Z Hermitian; the ratio is symmetrized host-side so the
    half-spectrum determines the output exactly).
    amp = |Z|; per-channel amp_sum AllReduced over the 8 cores.
    s = (ra_sym + mom*amp_sum_red) / amp        (symmetric real ratio)
    W = Z * s
    T2 = G @ W   (inverse row transform, G = conj(F))
    out[n_r,n_c] = sum_{k_c=0}^{256} w_k * Re(T2[n_r,k_c] G[k_c,n_c]) / N^2
      with w = [1, 2...2, 1] -- folded into the Gw constants.

Row transforms use a radix-2 split: stage 1 is a DIF butterfly on the 512
input rows followed by 256-point matmuls against the even/odd columns of F
(so the row-frequency axis lives in [evens, odds] permuted order, which the
host bakes into ra); stage 3 is the matching DIT inverse (even/odd G
columns) whose output butterfly is fused into the PSUM evacuation.

All matmuls f32r (~bf16 speed, ~1.5e-4 rel err). Data-stationary form:
out = lhsT.T @ rhs with lhsT = data chunk, rhs = constant block; each stage
flips the layout, so no transposes. Z and 1/amp round-trip DRAM in bf16.
Sharding: batch over 8 cores; amp mean via per-channel AllReduce.
"""
import sys

sys.path.insert(0, "/opt/trn_rl_repo")

import numpy as np

N_CORES = 8
B, C, H, W = 32, 3, 512, 512
B_LOC = B // N_CORES          # 4 batches per core
N_IMG = B_LOC * C             # 12 images per core
NBLK = H // 128               # 4 partition blocks
KC = 264                      # half-spectrum cols: 257 used + 7 zero pad
MOMENTUM = 0.1

_cached = {}


def _build():
    from concourse import bacc, tile, mybir

    f32 = mybir.dt.float32
    f32r = mybir.dt.float32r
    bf16 = mybir.dt.bfloat16
    Alu = mybir.AluOpType
    Act = mybir.ActivationFunctionType

    # Force every activation into the one table set covering
    # {copy, identity, square, ln, exp}: exactly one ACT table load
    # (the default per-function chooser thrashes sets, ~2.7us a reload).
    # Index order must be preserved, so other sets are stripped, not removed.
    from concourse import hw_specs as _hw
    if not getattr(_hw, "_ampnorm_patched", False):
        _orig_get_tables = _hw.get_activation_tables

        def _patched(module_arch):
            tabs = _orig_get_tables(module_arch)
            keep = "natural_log_exp_and_others"
            covered = tabs[keep]
            return {
                name: (fns if name == keep else (fns - covered))
                for name, fns in tabs.items()
            }

        _hw.get_activation_tables = _patched
        _hw._ampnorm_patched = True
        import concourse.bacc as _bacc_mod
        _bacc_mod.get_activation_tables = _patched

    nc = bacc.Bacc("TRN2", target_bir_lowering=False, debug=False,
                   num_devices=N_CORES)

    x_ext = nc.dram_tensor("x", [B_LOC, C, H, W], f32, kind="ExternalInput").ap()
    ra_ext = nc.dram_tensor("ra", [C, H, KC], f32, kind="ExternalInput").ap()
    mom_ext = nc.dram_tensor("mom", [128, 1], f32, kind="ExternalInput").ap()
    cdefs = {
        # stage1: B0 = F[0:256, 0::2], B1 = F[0:256, 1::2]  (radix-2 DIF)
        "B0r": [256, 256], "B0i": [256, 256], "B1r": [256, 256], "B1i": [256, 256],
        # stage2 rhs: half-width F
        "Frh": [H, KC], "Fih": [H, KC], "negFih": [H, KC],
        # stage3: Me/Mo = even/odd columns of G = conj(F), concatenated
        # [real | imag] so one matmul produces both halves (radix-2 DIT)
        "MeRI": [256, 512], "MeIR": [256, 512],
        "MoRI": [256, 512], "MoIR": [256, 512],
        # stage4: symmetry-weighted inverse column matrices (incl 1/N^2)
        "Gw1": [KC, W], "Gw2": [KC, W],
    }
    consts_ext = {
        name: nc.dram_tensor(name, shp, f32, kind="ExternalInput").ap()
        for name, shp in cdefs.items()
    }
    out_ext = nc.dram_tensor("out", [B_LOC, C, H, W], f32, kind="ExternalOutput").ap()

    q_slices = [(0, 128), (128, 256), (256, KC)]  # k_c chunks (2 full + slab)

    with tile.TileContext(nc) as tc:
        with (
            tc.tile_pool(name="const", bufs=1) as constp,
            tc.tile_pool(name="accum", bufs=1) as accp,
            tc.tile_pool(name="stage", bufs=1) as stagep,
            tc.tile_pool(name="work", bufs=2) as workp,
            tc.tile_pool(name="psum", bufs=3, space="PSUM") as psump,
            tc.tile_pool(name="psum1", bufs=1, space="PSUM") as psump1,
            tc.tile_pool(name="dram", bufs=1, space="DRAM") as dramp,
        ):
            # ---- constants: DMA f32, round to f32r on gpsimd ----
            cst = {}
            for name, shp in cdefs.items():
                tiles = []
                nb = (shp[0] + 127) // 128
                for k in range(nb):
                    p0 = k * 128
                    p1 = min(shp[0], p0 + 128)
                    stg = workp.tile([p1 - p0, shp[1]], f32,
                                     name=f"cstg_{name}_{k}", tag="cstg")
                    nc.sync.dma_start(stg[:], consts_ext[name][p0:p1, :])
                    t = constp.tile([p1 - p0, shp[1]], f32r, name=f"c_{name}_{k}")
                    nc.gpsimd.tensor_copy(t[:], stg[:])
                    tiles.append(t)
                cst[name] = tiles

            mom_t = constp.tile([128, 1], f32, name="mom_t")
            nc.sync.dma_start(mom_t[:], mom_ext[:, :])

            # ---- per-channel amp_sum accumulators [128, 4, KC] ----
            amp_sum = {}
            for c in range(C):
                t = accp.tile([128, NBLK, KC], f32, name=f"asum_{c}")
                nc.gpsimd.memset(t[:], 0.0)
                amp_sum[c] = t

            # ---- DRAM scratch (bf16) + collective bounces ----
            zscr = dramp.tile([N_IMG, 2, H, KC], bf16, name="zscr")
            iscr = dramp.tile([N_IMG, H, KC], bf16, name="iscr")
            ar_in = [dramp.tile([H, KC], f32, name=f"ar_in_{c}") for c in range(C)]
            ar_out = [
                dramp.tile([H, KC], f32, name=f"ar_out_{c}", addr_space="Shared")
                for c in range(C)
            ]

            def blocked(ap):  # [m*128+p, j] dram view -> [p, m, j]
                return ap.rearrange("(m p) j -> p m j", p=128)

            # ===== PASS 1 (per channel): forward + amp accumulation =====
            def emit_p1(c):
                for b in range(B_LOC):
                    img = b * C + c
                    xstg = workp.tile([128, NBLK, W], f32, name="xstg", tag="xstg")
                    nc.sync.dma_start(xstg[:], blocked(x_ext[b, c]))
                    # radix-2 DIF butterfly over rows (writes f32r directly)
                    y = {}
                    for (nm, j, op) in (("y0a", 0, Alu.add), ("y0b", 1, Alu.add),
                                        ("y1a", 0, Alu.subtract), ("y1b", 1, Alu.subtract)):
                        t = workp.tile([128, W], f32r, name=nm, tag=nm)
                        nc.vector.tensor_tensor(t[:], xstg[:, j, :], xstg[:, j + 2, :], op)
                        y[nm] = t
                    # stage 1: U^T[:, evens] = y0 @ B0; U^T[:, odds] = y1 @ B1
                    ur, ui = [], []
                    for m in range(NBLK):
                        ms = slice(m * 128, (m + 1) * 128)
                        psr = psump.tile([128, W], f32, name="ps1r", tag="psr")
                        psi = psump.tile([128, W], f32, name="ps1i", tag="psi")
                        for half, ya, yb, br, bi in (
                            (slice(0, 256), y["y0a"], y["y0b"], "B0r", "B0i"),
                            (slice(256, 512), y["y1a"], y["y1b"], "B1r", "B1i"),
                        ):
                            nc.tensor.matmul(psr[:, half], ya[:, ms], cst[br][0][:],
                                             start=True, stop=False)
                            nc.tensor.matmul(psr[:, half], yb[:, ms], cst[br][1][:],
                                             start=False, stop=True)
                            nc.tensor.matmul(psi[:, half], ya[:, ms], cst[bi][0][:],
                                             start=True, stop=False)
                            nc.tensor.matmul(psi[:, half], yb[:, ms], cst[bi][1][:],
                                             start=False, stop=True)
                        tr = stagep.tile([128, W], f32r, name=f"ur_{m}", tag=f"sr_{m}")
                        ti = stagep.tile([128, W], f32r, name=f"ui_{m}", tag=f"si_{m}")
                        nc.scalar.copy(tr[:], psr[:])
                        nc.vector.tensor_copy(ti[:], psi[:])
                        ur.append(tr)
                        ui.append(ti)
                    # stage 2: Z = U @ F_half  [k_r-part (permuted), k_c)
                    zrs = workp.tile([128, NBLK, KC], bf16, name="zrs", tag="zrs")
                    zis = workp.tile([128, NBLK, KC], bf16, name="zis", tag="zis")
                    invs = workp.tile([128, NBLK, KC], bf16, name="invs", tag="invs")
                    for m in range(NBLK):
                        ms = slice(m * 128, (m + 1) * 128)
                        psr = psump.tile([128, KC], f32, name="ps2r", tag="psr")
                        psi = psump.tile([128, KC], f32, name="ps2i", tag="psi")
                        for k in range(NBLK):
                            nc.tensor.matmul(psr[:], ur[k][:, ms], cst["Frh"][k][:],
                                             start=(k == 0), stop=False)
                        for k in range(NBLK):
                            nc.tensor.matmul(psr[:], ui[k][:, ms], cst["negFih"][k][:],
                                             start=False, stop=(k == NBLK - 1))
                        for k in range(NBLK):
                            nc.tensor.matmul(psi[:], ur[k][:, ms], cst["Fih"][k][:],
                                             start=(k == 0), stop=False)
                        for k in range(NBLK):
                            nc.tensor.matmul(psi[:], ui[k][:, ms], cst["Frh"][k][:],
                                             start=False, stop=(k == NBLK - 1))
                        nc.vector.tensor_copy(zrs[:, m, :], psr[:])
                        nc.vector.tensor_copy(zis[:, m, :], psi[:])
                        sq1 = workp.tile([128, KC], f32, name="sq1", tag="sq1")
                        sq2 = workp.tile([128, KC], f32, name="sq2", tag="sq2")
                        nc.vector.tensor_tensor(sq1[:], zrs[:, m, :], zrs[:, m, :],
                                                Alu.mult)
                        nc.scalar.square(sq2[:], psi[:])
                        a2 = workp.tile([128, KC], f32, name="a2", tag="a2")
                        nc.vector.scalar_tensor_tensor(
                            a2[:], sq1[:], 1e-30, sq2[:],
                            op0=Alu.add, op1=Alu.add)
                        lna = workp.tile([128, KC], f32, name="lna", tag="lna")
                        nc.scalar.activation(lna[:], a2[:], Act.Ln)
                        am = workp.tile([128, KC], f32, name="am", tag="am")
                        nc.scalar.activation(am[:], lna[:], Act.Exp, scale=0.5)
                        nc.scalar.activation(invs[:, m, :], lna[:], Act.Exp, scale=-0.5)
                        nc.gpsimd.tensor_add(amp_sum[c][:, m, :], amp_sum[c][:, m, :],
                                             am[:])
                    nc.sync.dma_start(blocked(zscr[img, 0]), zrs[:])
                    nc.sync.dma_start(blocked(zscr[img, 1]), zis[:])
                    nc.sync.dma_start(blocked(iscr[img]), invs[:])
                nc.sync.dma_start(blocked(ar_in[c][:, :]), amp_sum[c][:])
                nc.gpsimd.collective_compute(
                    "AllReduce",
                    Alu.add,
                    replica_groups=[list(range(N_CORES))],
                    ins=[ar_in[c].opt()],
                    outs=[ar_out[c].opt()],
                )

            # ===== PASS 2 (per channel): ratio + inverse =====
            # ar_wait[c]: virtual-time floor (ms) keeping the collective's
            # consumers out of the engine/DMA queues until the AllReduce is
            # done -- otherwise they head-of-line-block the whole pipeline.
            ar_wait = {0: 0.145, 1: 0.200, 2: 0.255}

            def emit_p2(c):
                with tc.tile_wait_until(ar_wait[c]):
                    red = workp.tile([128, NBLK, KC], f32, name="red", tag="red")
                    nc.sync.dma_start(red[:], blocked(ar_out[c][:, :]))
                    rat = stagep.tile([128, NBLK, KC], f32, name="rat", tag="rat")
                    nc.sync.dma_start(rat[:], blocked(ra_ext[c]))
                    numer = amp_sum[c]
                    nc.vector.scalar_tensor_tensor(
                        numer[:], red[:], mom_t[:, 0:1], rat[:],
                        op0=Alu.mult, op1=Alu.add)
                for b in range(B_LOC):
                    img = b * C + c
                    zrl = workp.tile([128, NBLK, KC], bf16, name="zrl", tag="zrl")
                    zil = workp.tile([128, NBLK, KC], bf16, name="zil", tag="zil")
                    invl = workp.tile([128, NBLK, KC], bf16, name="invl", tag="invl")
                    nc.sync.dma_start(zrl[:], blocked(zscr[img, 0]))
                    nc.sync.dma_start(zil[:], blocked(zscr[img, 1]))
                    nc.sync.dma_start(invl[:], blocked(iscr[img]))
                    twr, twi = [], []
                    for m in range(NBLK):
                        rn = workp.tile([128, KC], f32, name="rn", tag="rn")
                        nc.vector.tensor_mul(rn[:], numer[:, m, :], invl[:, m, :])
                        wr_t = stagep.tile([128, KC], f32r, name=f"wr_{m}", tag=f"sr_{m}")
                        wi_t = stagep.tile([128, KC], f32r, name=f"wi_{m}", tag=f"si_{m}")
                        nc.vector.tensor_mul(wr_t[:], zrl[:, m, :], rn[:])
                        nc.vector.tensor_mul(wi_t[:], zil[:, m, :], rn[:])
                        twr.append(wr_t)
                        twi.append(wi_t)
                    # stage 3 (radix-2 DIT over permuted k_r):
                    #   E = W_even^T Me, O = W_odd^T Mo;  T2^T = [E+O | E-O]
                    t2r, t2i = [], []
                    for qi, (q0, q1) in enumerate(q_slices):
                        qs = slice(q0, q1)
                        qn = q1 - q0
                        psE = psump.tile([qn, 512], f32, name="psE", tag="psr")
                        psO = psump.tile([qn, 512], f32, name="psO", tag="psi")
                        for k in range(2):
                            nc.tensor.matmul(psE[:], twr[k][:, qs], cst["MeRI"][k][:],
                                             start=(k == 0), stop=False)
                        for k in range(2):
                            nc.tensor.matmul(psE[:], twi[k][:, qs], cst["MeIR"][k][:],
                                             start=False, stop=(k == 1))
                        for k in range(2):
                            nc.tensor.matmul(psO[:], twr[k + 2][:, qs], cst["MoRI"][k][:],
                                             start=(k == 0), stop=False)
                        for k in range(2):
                            nc.tensor.matmul(psO[:], twi[k + 2][:, qs], cst["MoIR"][k][:],
                                             start=False, stop=(k == 1))
                        er = workp.tile([qn, 256], f32r, name="er", tag="er")
                        ei = workp.tile([qn, 256], f32r, name="ei", tag="ei")
                        nc.scalar.copy(er[:], psE[:, 0:256])
                        nc.scalar.copy(ei[:], psE[:, 256:512])
                        rt = stagep.tile([qn, 256], f32r, name=f"t2rt_{qi}", tag=f"t2rt_{qi}")
                        rb = stagep.tile([qn, 256], f32r, name=f"t2rb_{qi}", tag=f"t2rb_{qi}")
                        it_ = stagep.tile([qn, 256], f32r, name=f"t2it_{qi}", tag=f"t2it_{qi}")
                        ib = stagep.tile([qn, 256], f32r, name=f"t2ib_{qi}", tag=f"t2ib_{qi}")
                        nc.vector.tensor_add(rt[:], er[:], psO[:, 0:256])
                        nc.vector.tensor_sub(rb[:], er[:], psO[:, 0:256])
                        nc.vector.tensor_add(it_[:], ei[:], psO[:, 256:512])
                        nc.vector.tensor_sub(ib[:], ei[:], psO[:, 256:512])
                        t2r.append((rt, rb))
                        t2i.append((it_, ib))
                    # stage 4: out = sum_q T2r^T Gw1 + T2i^T Gw2
                    ostg = workp.tile([128, NBLK, W], f32, name="ostg", tag="ostg")
                    for m in range(NBLK):
                        half = m // 2          # 0: n_r in [0,256) -> top
                        ms = slice((m % 2) * 128, (m % 2) * 128 + 128)
                        pso = psump1.tile([128, W], f32, name="ps4", tag="ps4")
                        for qi in range(3):
                            nc.tensor.matmul(pso[:], t2r[qi][half][:, ms],
                                             cst["Gw1"][qi][:],
                                             start=(qi == 0), stop=False)
                        for qi in range(3):
                            nc.tensor.matmul(pso[:], t2i[qi][half][:, ms],
                                             cst["Gw2"][qi][:],
                                             start=False, stop=(qi == 2))
                        nc.scalar.copy(ostg[:, m, :], pso[:])
                    nc.sync.dma_start(blocked(out_ext[b, c]), ostg[:])


            # Interleave: keep each collective's consumers ~one channel of
            # work downstream in every engine stream, so nothing queues
            # behind an in-flight AllReduce (the scheduler otherwise hoists
            # pass-2 ops early and stalls the whole machine ~30us per AR).
            emit_p1(0)
            emit_p1(1)
            emit_p1(2)
            emit_p2(0)
            emit_p2(1)
            emit_p2(2)
    nc.compile()
    return nc


def _host_inputs(x, running_amp):
    j = np.arange(H)
    theta = -2.0 * np.pi * np.outer(j, j) / H
    Fc = np.exp(1j * theta)           # F[n,k] = w^{nk}
    Gc = np.conj(Fc)                  # G[n,k] = w^{-nk}

    def halfpad(M):
        out = np.zeros((H, KC), np.float32)
        out[:, :257] = M[:, :257]
        return out

    # stage1 radix-2: B0/B1 = even/odd columns of F, top 256 rows
    B0 = Fc[0:256, 0::2]
    B1 = Fc[0:256, 1::2]
    # stage3 radix-2 DIT: even/odd columns of G restricted per derivation
    n256 = np.arange(256)
    Me = np.exp(2j * np.pi * np.outer(n256, n256) / 256.0)      # G[n,2k'] on n<256
    Mo = np.exp(2j * np.pi * np.outer(2 * n256 + 1, n256) / 512.0)  # [k',n']
    MeRI = np.concatenate([Me.real, Me.imag], axis=1)
    MeIR = np.concatenate([-Me.imag, Me.real], axis=1)
    MoRI = np.concatenate([Mo.real, Mo.imag], axis=1)
    MoIR = np.concatenate([-Mo.imag, Mo.real], axis=1)

    # stage-4 weights: w_k in {1,2}, zero on pad; scaled by 1/N^2
    wgt = np.zeros(KC)
    wgt[0] = 1.0
    wgt[1:256] = 2.0
    wgt[256] = 1.0
    Gw1 = np.zeros((KC, W), np.float32)
    Gw2 = np.zeros((KC, W), np.float32)
    Gw1[:257] = (wgt[:257, None] * Gc[:257, :].real / (H * W)).astype(np.float32)
    Gw2[:257] = (-wgt[:257, None] * Gc[:257, :].imag / (H * W)).astype(np.float32)

    f32 = np.float32
    consts = {
        "B0r": B0.real.astype(f32), "B0i": B0.imag.astype(f32),
        "B1r": B1.real.astype(f32), "B1i": B1.imag.astype(f32),
        "Frh": halfpad(Fc.real.astype(f32)), "Fih": halfpad(Fc.imag.astype(f32)),
        "negFih": halfpad((-Fc.imag).astype(f32)),
        "MeRI": MeRI.astype(f32), "MeIR": MeIR.astype(f32),
        "MoRI": MoRI.astype(f32), "MoIR": MoIR.astype(f32),
        "Gw1": Gw1, "Gw2": Gw2,
    }

    perm_kr = np.concatenate([np.arange(0, H, 2), np.arange(1, H, 2)])
    if abs(float(running_amp.sum())) == 0.0:
        ra_half = np.zeros((C, H, KC), np.float32)
        mom_eff = 1.0 / B
    else:
        ra_s = np.fft.ifftshift(running_amp, axes=(-2, -1)).astype(np.float64)
        ra_rev = ra_s[:, (-np.arange(H)) % H][:, :, (-np.arange(W)) % W]
        ra_sym = (1.0 - MOMENTUM) * 0.5 * (ra_s + ra_rev)
        ra_half = np.zeros((C, H, KC), np.float32)
        ra_half[:, :, :257] = ra_sym[:, perm_kr][:, :, :257].astype(np.float32)
        mom_eff = MOMENTUM / B
    mom = np.full((128, 1), mom_eff, np.float32)

    in_maps = []
    for i in range(N_CORES):
        m = {"x": np.ascontiguousarray(x[i * B_LOC:(i + 1) * B_LOC]),
             "ra": ra_half, "mom": mom}
        m.update(consts)
        in_maps.append(m)
    return in_maps


def kernel(x: np.ndarray, running_amp: np.ndarray) -> np.ndarray:
    from concourse.bass_utils import run_bass_kernel_spmd

    if "nc" not in _cached:
        _cached["nc"] = _build()
    nc = _cached["nc"]
    in_maps = _host_inputs(np.asarray(x, np.float32),
                           np.asarray(running_amp, np.float32))
    res = run_bass_kernel_spmd(nc, in_maps, list(range(N_CORES)))
    out = np.concatenate([res.results[i]["out"] for i in range(N_CORES)], axis=0)
    return out.astype(np.float32)



# revision 23
# speedup vs baseline: 1.8619x; 1.8619x over previous
"""AmpNorm Trainium2 kernel: FFT-domain amplitude normalization.

reference semantics:
    fft = fft2(x); amp = fftshift(|fft|); pha = angle(fft)
    amp_mean = mean(amp, axis=0)
    new_amp = (1-m)*running_amp + m*amp_mean     (EMA branch; init branch if sum==0)
    out = real(ifft2(ifftshift(new_amp) * exp(i*pha)))

Device formulation (per [512,512] image; shifts absorbed on host):
    Z = F @ X @ F computed for the k_c half-spectrum [0, 256] (x real =>
    Z Hermitian; the ratio is symmetrized host-side so the half determines
    the output exactly). P = Z/|Z| is stored (phase); out is rebuilt as
    ifft2 of numer * P where numer = ra_sym + mom * AllReduce(sum |Z|).

All four matmul stages are radix-2 split so every PE pass contracts over
<=256 rows (2 chunks of 128):
  stage 1 (rows):   DIF butterfly on input rows, then [B0|B0i]/[B1|B1i]
                    merged-rhs matmuls -> U^T blocks [Ur|Ui] per k_r parity.
  stage 2 (cols):   DIF butterfly over n_c -- applied to the INPUT (both
                    butterflies commute with stage 1, so they run as cheap
                    pre-ops on x), then merged [Be_r|Be_i] (129 even k_c
                    incl nyquist) and [Bo_r|Bo_i] (128 odd k_c) matmuls.
                    k_c layout: [evens 0:129 | odds 129:257].
  stage 3 (rows⁻¹): radix-2 DIT (Me/Mo on k_r parity chunks), butterfly
                    fused into PSUM evacuation. k_c q-blocks {evens sans
                    nyq, odds}, 128 wide each.
  stage 4 (cols⁻¹): two-chunk contraction against Gw1/Gw2; the k_c=256
                    (nyquist) rank-1 term enters the same PSUM group as a
                    K=4 matmul of the per-channel T2nyq rows against a
                    per-image (-1)^{n_c} selector row (Sg{b}).

All matmuls bf16 (weights get the fast-weight-load path; loads hide under
the streams). DVE pointwise work runs 16-bit in+out and is batched into
wide multi-m ops (per-op fixed cost dominates). P round-trips DRAM in
bf16; the amp AllReduce payload is bf16. Sharding: batch over 8 cores;
amp mean via per-channel AllReduce.
"""
import sys

sys.path.insert(0, "/opt/trn_rl_repo")

import numpy as np

N_CORES = 8
B, C, H, W = 32, 3, 512, 512
B_LOC = B // N_CORES          # 4 batches per core
N_IMG = B_LOC * C             # 12 images per core
NBLK = H // 128               # 4 partition blocks
KC = 257                      # half-spectrum cols: [evens 0..256 | odds]
NE = 129                      # even k_c count (incl nyquist)
NO = 128                      # odd k_c count
MOMENTUM = 0.1

# bf16 constants, packed column-wise into one [128, CST_COLS] tensor.
# name -> list of [rows, width] per 128-row chunk (order = chunk index).
CDEFS = {
    # stage1 merged rhs: CA = [B0r|B0i], CB = [B1r|B1i] (radix-2 DIF rows)
    "CA": [256, 512], "CB": [256, 512],
    # stage2 merged rhs (radix-2 DIF over n_c):
    #   CE = [Ber|Bei], CEm = [-Bei|Ber]  (even k_c incl nyquist)
    #   CO = [Bor|Boi], COm = [-Boi|Bor]  (odd k_c; twiddle folded in)
    "CE": [256, 2 * NE], "CEm": [256, 2 * NE],
    "CO": [256, 2 * NO], "COm": [256, 2 * NO],
    # stage3: Me/Mo = even/odd columns of G = conj(F), concatenated
    # [real | imag] so one matmul produces both halves (radix-2 DIT)
    "MeRI": [256, 512], "MeIR": [256, 512],
    "MoRI": [256, 512], "MoIR": [256, 512],
    # stage4: weighted inverse column matrices (incl 1/N^2), rows in
    # [evens sans nyq | odds] order
    "Gw1": [256, W], "Gw2": [256, W],
    # nyquist side path: G rows in k_r-perm order, scaled by 1/N^2
    "GnR": [H, H], "GnI": [H, H],
    # per-image selector rows for the nyquist rank-1 add: Sg{b} is all
    # zeros except row b = (-1)^{n_c}
    "Sg0": [B_LOC, W], "Sg1": [B_LOC, W],
    "Sg2": [B_LOC, W], "Sg3": [B_LOC, W],
}


def _cst_layout():
    """name -> list of (col_offset, rows, width) per 128-row chunk."""
    chunks = {}
    off = 0
    for name, (r, wdt) in CDEFS.items():
        lst = []
        for p0 in range(0, r, 128):
            rows = min(128, r - p0)
            lst.append((off, rows, wdt))
            off += wdt
        chunks[name] = lst
    return chunks, off


_cached = {}


def _build():
    from concourse import bacc, tile, mybir

    f32 = mybir.dt.float32
    bf16 = mybir.dt.bfloat16
    Alu = mybir.AluOpType
    Act = mybir.ActivationFunctionType

    # Force every activation into the one table set covering
    # {copy, identity, square, ln, exp}: exactly one ACT table load
    # (the default per-function chooser thrashes sets, ~2.7us a reload).
    from concourse import hw_specs as _hw
    if not getattr(_hw, "_ampnorm_patched", False):
        _orig_get_tables = _hw.get_activation_tables

        def _patched(module_arch):
            tabs = _orig_get_tables(module_arch)
            keep = "natural_log_exp_and_others"
            covered = tabs[keep]
            return {
                name: (fns if name == keep else (fns - covered))
                for name, fns in tabs.items()
            }

        _hw.get_activation_tables = _patched
        _hw._ampnorm_patched = True
        import concourse.bacc as _bacc_mod
        _bacc_mod.get_activation_tables = _patched

    nc = bacc.Bacc("TRN2", target_bir_lowering=False, debug=False,
                   num_devices=N_CORES)

    x_ext = nc.dram_tensor("x", [B_LOC, C, H, W], f32, kind="ExternalInput").ap()
    ra_ext = nc.dram_tensor("ra", [C, H, KC], f32, kind="ExternalInput").ap()
    mom_ext = nc.dram_tensor("mom", [128, 1], f32, kind="ExternalInput").ap()
    # All bf16 constants are packed column-wise into ONE [128, CST_COLS]
    # tensor so startup is a single large DMA instead of ~40 descriptor
    # builds on the Sync engine. _cst_layout() maps name -> per-chunk
    # (col offset, rows, width); chunks with <128 rows sit in rows 0:r.
    cchunks, CST_COLS = _cst_layout()
    cst_ext = nc.dram_tensor("CST", [128, CST_COLS], bf16,
                             kind="ExternalInput").ap()
    out_ext = nc.dram_tensor("out", [B_LOC, C, H, W], f32, kind="ExternalOutput").ap()

    q_slices = [(0, 128), (NE, KC)]   # stage3/4 k_c blocks (even sans nyq, odd)

    with tile.TileContext(nc) as tc:
        with (
            tc.tile_pool(name="const", bufs=1) as constp,
            tc.tile_pool(name="accum", bufs=1) as accp,
            tc.tile_pool(name="stage", bufs=1) as stagep,
            tc.tile_pool(name="work", bufs=2) as workp,
            tc.tile_pool(name="psum", bufs=3, space="PSUM") as psump,
            tc.tile_pool(name="psum1", bufs=2, space="PSUM") as psump1,
            tc.tile_pool(name="dram", bufs=1, space="DRAM") as dramp,
        ):
            # ---- constants: two packed DMAs (hot = pass-1 consts first so
            # stage 1 isn't gated on the pass-2 constants) ----
            cbig = constp.tile([128, CST_COLS], bf16, name="cbig")
            hot_end = cchunks["COm"][-1][0] + cchunks["COm"][-1][2]
            nc.sync.dma_start(cbig[:, 0:hot_end], cst_ext[:, 0:hot_end])
            nc.sync.dma_start(cbig[:, hot_end:], cst_ext[:, hot_end:])
            cst = {
                name: [cbig[0:rows, o:o + wdt] for (o, rows, wdt) in lst]
                for name, lst in cchunks.items()
            }

            mom_t = constp.tile([128, 1], f32, name="mom_t")
            nc.sync.dma_start(mom_t[:], mom_ext[:, :])

            # ---- per-channel amp_sum accumulators [128, 4, KC] ----
            amp_sum = {}
            for c in range(C):
                t = accp.tile([128, NBLK, KC], f32, name=f"asum_{c}")
                nc.gpsimd.memset(t[:], 0.0)
                amp_sum[c] = t

            # ---- DRAM scratch (bf16) + collective bounces (bf16) ----
            # [C, B_LOC, ...] so each channel's nyquist gather is one DMA
            zscr = dramp.tile([C, B_LOC, H, 2, KC], bf16, name="zscr")
            ar_in = [dramp.tile([H, KC], bf16, name=f"ar_in_{c}") for c in range(C)]
            ar_out = [
                dramp.tile([H, KC], bf16, name=f"ar_out_{c}", addr_space="Shared")
                for c in range(C)
            ]

            def blocked(ap):  # [m*128+p, j] dram view -> [p, m, j]
                return ap.rearrange("(m p) j -> p m j", p=128)

            # ===== PASS 1 (per channel): forward + amp accumulation =====
            def emit_p1(c):
                for b in range(B_LOC):
                    xstg = workp.tile([128, NBLK, W], f32, name="xstg", tag="xstg")
                    nc.sync.dma_start(xstg[:], blocked(x_ext[b, c]))
                    # radix-2 DIF butterflies over BOTH axes on the input
                    # (row butterfly feeds stage 1's k_r parity, column
                    # butterfly feeds stage 2's k_c parity -- the two
                    # transforms commute so both fold into cheap pre-ops)
                    # packed row butterfly: y0/y1 hold both n-chunks (a,b)
                    y0 = workp.tile([128, 2, W], bf16, name="y0", tag="y0")
                    y1 = workp.tile([128, 2, W], bf16, name="y1", tag="y1")
                    nc.gpsimd.tensor_tensor(y0[:], xstg[:, 0:2, :],
                                            xstg[:, 2:4, :], Alu.add)
                    nc.vector.tensor_tensor(y1[:], xstg[:, 0:2, :],
                                            xstg[:, 2:4, :], Alu.subtract)
                    # packed column butterfly -> 4 tiles [128, 2(chunk), 256]
                    yb = {}
                    for (nm, src, op) in (("p0", y0, Alu.add), ("m0", y0, Alu.subtract),
                                          ("p1", y1, Alu.add), ("m1", y1, Alu.subtract)):
                        t = workp.tile([128, 2, 256], bf16, name=f"y{nm}",
                                       tag=f"y{nm}")
                        nc.vector.tensor_tensor(t[:], src[:, :, 0:256],
                                                src[:, :, 256:512], op)
                        yb[nm] = t
                    # stage 1: psUpE[mp] = [Up_r_e|Up_i_e] etc; evac to SBUF
                    ups = {}
                    for (key, yt, cn) in (
                        ("pE", yb["p0"], "CA"), ("pO", yb["p1"], "CB"),
                        ("mE", yb["m0"], "CA"), ("mO", yb["m1"], "CB"),
                    ):
                        for mp in range(2):
                            mps = slice(mp * 128, (mp + 1) * 128)
                            tag = "psr" if key[1] == "E" else "psi"
                            ps = psump.tile([128, W], f32, name=f"ps1{key}", tag=tag)
                            nc.tensor.matmul(ps[:], yt[:, 0, mps], cst[cn][0][:],
                                             start=True, stop=False)
                            nc.tensor.matmul(ps[:], yt[:, 1, mps], cst[cn][1][:],
                                             start=False, stop=True)
                            t = stagep.tile([128, W], bf16, name=f"u{key}{mp}",
                                            tag=f"u{key}{mp}")
                            if key[1] == "E":
                                nc.scalar.copy(t[:], ps[:])
                            else:
                                nc.vector.tensor_copy(t[:], ps[:])
                            ups[key + str(mp)] = t
                    # stage 2 matmuls + amp/phase evacuation. The pointwise
                    # amp chain is batched over PAIRS of m-blocks -- per-op
                    # fixed cost dominates DVE/ACT time, so fewer, wider ops.
                    ppack = workp.tile([128, NBLK, 2, KC], bf16, name="ppack",
                                       tag="ppack")
                    if b == B_LOC - 1:
                        # last image: the amp add writes the bf16 AllReduce
                        # input directly (keeps vector out of the AR chain)
                        acast = workp.tile([128, NBLK, KC], bf16, name="acast",
                                           tag="acast")
                    for mh in range(2):
                        zr2 = workp.tile([128, 2, KC], bf16, name="zr2", tag="zr2")
                        zi2 = workp.tile([128, 2, KC], bf16, name="zi2", tag="zi2")
                        for dm in range(2):
                            m = 2 * mh + dm
                            # k_r-evens chunks (m 0,1) live in *E tiles,
                            # odds (m 2,3) in *O; r cols [0:256), i [256:512)
                            sfx = "E" if m < 2 else "O"
                            mr = slice((m % 2) * 128, (m % 2) * 128 + 128)
                            mi = slice(256 + (m % 2) * 128,
                                       256 + (m % 2) * 128 + 128)
                            psZe = psump.tile([128, 2 * NE], f32, name="psZe",
                                              tag="psr")
                            psZo = psump.tile([128, 2 * NO], f32, name="psZo",
                                              tag="psi")
                            for mp in range(2):
                                nc.tensor.matmul(psZe[:],
                                                 ups["p" + sfx + str(mp)][:, mr],
                                                 cst["CE"][mp][:],
                                                 start=(mp == 0), stop=False)
                            for mp in range(2):
                                nc.tensor.matmul(psZe[:],
                                                 ups["p" + sfx + str(mp)][:, mi],
                                                 cst["CEm"][mp][:],
                                                 start=False, stop=(mp == 1))
                            for mp in range(2):
                                nc.tensor.matmul(psZo[:],
                                                 ups["m" + sfx + str(mp)][:, mr],
                                                 cst["CO"][mp][:],
                                                 start=(mp == 0), stop=False)
                            for mp in range(2):
                                nc.tensor.matmul(psZo[:],
                                                 ups["m" + sfx + str(mp)][:, mi],
                                                 cst["COm"][mp][:],
                                                 start=False, stop=(mp == 1))
                            nc.vector.tensor_copy(zr2[:, dm, 0:NE], psZe[:, 0:NE])
                            nc.vector.tensor_copy(zr2[:, dm, NE:KC], psZo[:, 0:NO])
                            nc.scalar.copy(zi2[:, dm, 0:NE], psZe[:, NE:2 * NE])
                            nc.scalar.copy(zi2[:, dm, NE:KC], psZo[:, NO:2 * NO])
                        sq1 = workp.tile([128, 2, KC], bf16, name="sq1", tag="sq1")
                        sq2 = workp.tile([128, 2, KC], bf16, name="sq2", tag="sq2")
                        nc.vector.tensor_tensor(sq1[:], zr2[:], zr2[:], Alu.mult)
                        nc.scalar.square(sq2[:], zi2[:])
                        a2 = workp.tile([128, 2, KC], bf16, name="a2", tag="a2")
                        nc.vector.scalar_tensor_tensor(
                            a2[:], sq1[:], 1e-30, sq2[:],
                            op0=Alu.add, op1=Alu.add)
                        lna = workp.tile([128, 2, KC], f32, name="lna", tag="lna")
                        nc.scalar.activation(lna[:], a2[:], Act.Ln)
                        am = workp.tile([128, 2, KC], f32, name="am", tag="am")
                        nc.scalar.activation(am[:], lna[:], Act.Exp, scale=0.5)
                        inv = workp.tile([128, 2, KC], bf16, name="inv", tag="inv")
                        nc.scalar.activation(inv[:], lna[:], Act.Exp, scale=-0.5)
                        nc.vector.tensor_mul(ppack[:, 2 * mh:2 * mh + 2, 0, :],
                                             zr2[:], inv[:])
                        nc.vector.tensor_mul(ppack[:, 2 * mh:2 * mh + 2, 1, :],
                                             zi2[:], inv[:])
                        dst = (acast if b == B_LOC - 1 else amp_sum[c])
                        nc.gpsimd.tensor_tensor(dst[:, 2 * mh:2 * mh + 2, :],
                                                amp_sum[c][:, 2 * mh:2 * mh + 2, :],
                                                am[:], Alu.add)
                    nc.sync.dma_start(
                        zscr[c, b].rearrange("(m p) t j -> p m (t j)", p=128),
                        ppack[:].rearrange("p m t j -> p m (t j)"))
                # schedule the amp cast + bounce DMA + trigger as early as
                # their deps allow -- otherwise the scheduler parks the
                # trigger behind the next channels' engine work and the
                # collective fires ~45us late
                with tc.high_priority():
                    nc.sync.dma_start(blocked(ar_in[c][:, :]), acast[:])
                    nc.gpsimd.collective_compute(
                        "AllReduce",
                        Alu.add,
                        replica_groups=[list(range(N_CORES))],
                        ins=[ar_in[c].opt()],
                        outs=[ar_out[c].opt()],
                    )

            # ===== PASS 2 (per channel): ratio + inverse =====
            # ar_wait[c]: virtual-time floor (ms) keeping the collective's
            # consumers out of the engine/DMA queues until the AllReduce is
            # done -- otherwise they head-of-line-block the whole pipeline.
            ar_wait = {0: 0.120, 1: 0.175, 2: 0.235}

            def emit_p2(c):
                with tc.tile_wait_until(ar_wait[c]):
                    red = workp.tile([128, NBLK, KC], bf16, name="red", tag="red")
                    nc.sync.dma_start(red[:], blocked(ar_out[c][:, :]))
                    rat = stagep.tile([128, NBLK, KC], f32, name="rat", tag="rat")
                    nc.sync.dma_start(rat[:], blocked(ra_ext[c]))
                    numer = stagep.tile([128, NBLK, KC], bf16, name=f"numer_{c}",
                                        tag=f"numer_{c}")
                    nc.vector.scalar_tensor_tensor(
                        numer[:], red[:], mom_t[:, 0:1], rat[:],
                        op0=Alu.mult, op1=Alu.add)
                    # ---- nyquist side path (batched over B_LOC images) ----
                    # layout [p, t, b, m] so each t is one gather DMA
                    pn = workp.tile([128, 2, B_LOC, NBLK], bf16, name="pn", tag="pn")
                    for t in range(2):
                        nc.sync.dma_start(
                            pn[:, t],
                            zscr[c][:, :, t, 128:129]
                            .rearrange("b (m p) o -> p b (m o)", p=128))
                    nq = workp.tile([128, NBLK, 1], f32, name="nq", tag="nq")
                    nc.scalar.copy(nq[:], numer[:, :, 128:129])
                    wn = workp.tile([128, 2, B_LOC, NBLK], bf16, name="wn", tag="wn")
                    for m in range(NBLK):
                        nc.vector.tensor_scalar_mul(
                            wn[:, :, :, m], pn[:, :, :, m], nq[:, m, 0:1])
                    # T2nyq rows for the channel's B_LOC images (data-
                    # stationary: out [B_LOC, n_r])
                    psn = psump1.tile([B_LOC, W], f32, name="psn", tag="ps4")
                    for k in range(NBLK):
                        nc.tensor.matmul(psn[:], wn[:, 0, :, k], cst["GnR"][k][:],
                                         start=(k == 0), stop=False)
                    for k in range(NBLK):
                        nc.tensor.matmul(psn[:], wn[:, 1, :, k], cst["GnI"][k][:],
                                         start=False, stop=(k == NBLK - 1))
                    tnyR = stagep.tile([B_LOC, W], bf16, name="tnyR", tag="tnyR")
                    nc.scalar.copy(tnyR[:], psn[:])
                for b in range(B_LOC):
                    ppl = workp.tile([128, NBLK, 2, KC], bf16, name="ppl", tag="ppl")
                    # issue pass-2 bulk DMAs from the (tail-idle) Pool queue
                    # so descriptor builds don't pile up on Sync
                    nc.gpsimd.dma_start(
                        ppl[:].rearrange("p m t j -> p m (t j)"),
                        zscr[c, b].rearrange("(m p) t j -> p m (t j)", p=128))
                    twp_r = stagep.tile([128, NBLK, KC], bf16, name="twp_r",
                                        tag="twp_r")
                    twp_i = stagep.tile([128, NBLK, KC], bf16, name="twp_i",
                                        tag="twp_i")
                    nc.vector.tensor_mul(twp_r[:], ppl[:, :, 0, :], numer[:])
                    nc.vector.tensor_mul(twp_i[:], ppl[:, :, 1, :], numer[:])
                    twr = [twp_r[:, m, :] for m in range(NBLK)]
                    twi = [twp_i[:, m, :] for m in range(NBLK)]
                    # stage 3 (radix-2 DIT over permuted k_r):
                    #   E = We^T Me, O = Wo^T Mo;  T2^T = [E+O | E-O]
                    t2r, t2i = [], []
                    for qi, (q0, q1) in enumerate(q_slices):
                        qs = slice(q0, q1)
                        psE = psump.tile([128, 512], f32, name="psE", tag="psr")
                        psO = psump.tile([128, 512], f32, name="psO", tag="psi")
                        for k in range(2):
                            nc.tensor.matmul(psE[:], twr[k][:, qs], cst["MeRI"][k][:],
                                             start=(k == 0), stop=False)
                        for k in range(2):
                            nc.tensor.matmul(psE[:], twi[k][:, qs], cst["MeIR"][k][:],
                                             start=False, stop=(k == 1))
                        for k in range(2):
                            nc.tensor.matmul(psO[:], twr[k + 2][:, qs], cst["MoRI"][k][:],
                                             start=(k == 0), stop=False)
                        for k in range(2):
                            nc.tensor.matmul(psO[:], twi[k + 2][:, qs], cst["MoIR"][k][:],
                                             start=False, stop=(k == 1))
                        er = workp.tile([128, 256], bf16, name="er", tag="er")
                        ei = workp.tile([128, 256], bf16, name="ei", tag="ei")
                        nc.scalar.copy(er[:], psE[:, 0:256])
                        nc.scalar.copy(ei[:], psE[:, 256:512])
                        rt = stagep.tile([128, 256], bf16, name=f"t2rt_{qi}", tag=f"t2rt_{qi}")
                        rb = stagep.tile([128, 256], bf16, name=f"t2rb_{qi}", tag=f"t2rb_{qi}")
                        it_ = stagep.tile([128, 256], bf16, name=f"t2it_{qi}", tag=f"t2it_{qi}")
                        ib = stagep.tile([128, 256], bf16, name=f"t2ib_{qi}", tag=f"t2ib_{qi}")
                        nc.vector.tensor_add(rt[:], er[:], psO[:, 0:256])
                        nc.vector.tensor_sub(rb[:], er[:], psO[:, 0:256])
                        nc.vector.tensor_add(it_[:], ei[:], psO[:, 256:512])
                        nc.vector.tensor_sub(ib[:], ei[:], psO[:, 256:512])
                        t2r.append((rt, rb))
                        t2i.append((it_, ib))
                    # stage 4: out = sum_q T2r^T Gw1 + T2i^T Gw2, plus the
                    # nyquist rank-1 term as a K=4 selector matmul
                    ostg = workp.tile([128, NBLK, W], f32, name="ostg", tag="ostg")
                    for m in range(NBLK):
                        half = m // 2          # 0: n_r in [0,256) -> top
                        ms = slice(m * 128, (m + 1) * 128)
                        msh = slice((m % 2) * 128, (m % 2) * 128 + 128)
                        pso = psump1.tile([128, W], f32, name="ps4", tag="ps4")
                        for qi in range(2):
                            nc.tensor.matmul(pso[:], t2r[qi][half][:, msh],
                                             cst["Gw1"][qi][:],
                                             start=(qi == 0), stop=False)
                        nc.tensor.matmul(pso[:], tnyR[:, ms],
                                         cst[f"Sg{b}"][0][:],
                                         start=False, stop=False)
                        for qi in range(2):
                            nc.tensor.matmul(pso[:], t2i[qi][half][:, msh],
                                             cst["Gw2"][qi][:],
                                             start=False, stop=(qi == 1))
                        if m % 2 == 0:
                            nc.scalar.copy(ostg[:, m, :], pso[:])
                        else:
                            nc.vector.tensor_copy(ostg[:, m, :], pso[:])
                    nc.gpsimd.dma_start(blocked(out_ext[b, c]), ostg[:])

            # Interleave: keep each collective's consumers ~one channel of
            # work downstream in every engine stream, so nothing queues
            # behind an in-flight AllReduce.
            emit_p1(0)
            emit_p1(1)
            emit_p1(2)
            emit_p2(0)
            emit_p2(1)
            emit_p2(2)
    nc.compile()
    return nc


def _host_inputs(x, running_amp):
    import ml_dtypes
    bf16 = ml_dtypes.bfloat16
    f32 = np.float32

    j = np.arange(H)
    theta = -2.0 * np.pi * np.outer(j, j) / H
    Fc = np.exp(1j * theta)           # F[n,k] = w^{nk}
    Gc = np.conj(Fc)                  # G[n,k] = w^{-nk}

    # stage1: B0/B1 = even/odd columns of F, top 256 rows; merged [r|i]
    B0 = Fc[0:256, 0::2]
    B1 = Fc[0:256, 1::2]
    CA = np.concatenate([B0.real, B0.imag], axis=1).astype(bf16)
    CB = np.concatenate([B1.real, B1.imag], axis=1).astype(bf16)

    # stage2: radix-2 DIF over n_c. Be[n,k]=W^(n*2k) (even k_c incl nyq),
    # Bo[n,k]=W^(n*(2k+1)) (odd k_c, twiddle folded)
    n = np.arange(256)[:, None]
    ke = np.arange(NE)[None, :]
    ko = np.arange(NO)[None, :]
    Be = np.exp(-2j * np.pi * n * (2 * ke) / H)
    Bo = np.exp(-2j * np.pi * n * (2 * ko + 1) / H)
    CE = np.concatenate([Be.real, Be.imag], axis=1).astype(bf16)
    CEm = np.concatenate([-Be.imag, Be.real], axis=1).astype(bf16)
    CO = np.concatenate([Bo.real, Bo.imag], axis=1).astype(bf16)
    COm = np.concatenate([-Bo.imag, Bo.real], axis=1).astype(bf16)

    # stage3 radix-2 DIT: even/odd columns of G restricted per derivation
    n256 = np.arange(256)
    Me = np.exp(2j * np.pi * np.outer(n256, n256) / 256.0)
    Mo = np.exp(2j * np.pi * np.outer(2 * n256 + 1, n256) / 512.0)  # [k',n']
    MeRI = np.concatenate([Me.real, Me.imag], axis=1).astype(bf16)
    MeIR = np.concatenate([-Me.imag, Me.real], axis=1).astype(bf16)
    MoRI = np.concatenate([Mo.real, Mo.imag], axis=1).astype(bf16)
    MoIR = np.concatenate([-Mo.imag, Mo.real], axis=1).astype(bf16)

    # k_c column order: [evens incl nyq | odds]
    cols = np.concatenate([np.arange(0, 257, 2), np.arange(1, 256, 2)])
    cols_noq = np.concatenate([np.arange(0, 256, 2), np.arange(1, 256, 2)])

    # stage-4 weights: w=1 for k_c=0, else 2 (nyquist handled separately)
    wgt = np.where(cols_noq == 0, 1.0, 2.0)
    Gsel = Gc[cols_noq, :]
    Gw1 = (wgt[:, None] * Gsel.real / (H * W)).astype(bf16)
    Gw2 = (-wgt[:, None] * Gsel.imag / (H * W)).astype(bf16)

    perm_kr = np.concatenate([np.arange(0, H, 2), np.arange(1, H, 2)])
    # nyquist path: T2nyq = sum_kr G[n_r,k_r] W[k_r]; G symmetric; rows in
    # k_r-perm order; 1/N^2 and w=1 folded in. Re only: Gr*Wr - Gi*Wi.
    Gn = Gc[perm_kr, :]
    GnR = (Gn.real / (H * W)).astype(bf16)
    GnI = (-Gn.imag / (H * W)).astype(bf16)
    sgnrow = np.where(np.arange(W) % 2 == 0, 1.0, -1.0)

    cvals = {
        "CA": CA, "CB": CB, "CE": CE, "CEm": CEm, "CO": CO, "COm": COm,
        "MeRI": MeRI, "MeIR": MeIR, "MoRI": MoRI, "MoIR": MoIR,
        "Gw1": Gw1, "Gw2": Gw2, "GnR": GnR, "GnI": GnI,
    }
    for b in range(B_LOC):
        sg = np.zeros((B_LOC, W), np.float32)
        sg[b] = sgnrow
        cvals[f"Sg{b}"] = sg.astype(bf16)

    # pack all bf16 constants column-wise into one [128, CST_COLS] array
    cchunks, cst_cols = _cst_layout()
    CST = np.zeros((128, cst_cols), bf16)
    for name, lst in cchunks.items():
        arr = cvals[name]
        p0 = 0
        for (o, rows, wdt) in lst:
            CST[0:rows, o:o + wdt] = arr[p0:p0 + rows, :]
            p0 += rows
    consts = {"CST": CST}

    if abs(float(running_amp.sum())) == 0.0:
        ra_half = np.zeros((C, H, KC), np.float32)
        mom_eff = 1.0 / B
    else:
        ra_s = np.fft.ifftshift(running_amp, axes=(-2, -1)).astype(np.float64)
        ra_rev = ra_s[:, (-np.arange(H)) % H][:, :, (-np.arange(W)) % W]
        ra_sym = (1.0 - MOMENTUM) * 0.5 * (ra_s + ra_rev)
        ra_half = ra_sym[:, perm_kr][:, :, cols].astype(np.float32)
        mom_eff = MOMENTUM / B
    mom = np.full((128, 1), mom_eff, np.float32)

    in_maps = []
    for i in range(N_CORES):
        m = {"x": np.ascontiguousarray(x[i * B_LOC:(i + 1) * B_LOC]),
             "ra": ra_half, "mom": mom}
        m.update(consts)
        in_maps.append(m)
    return in_maps


def kernel(x: np.ndarray, running_amp: np.ndarray) -> np.ndarray:
    from concourse.bass_utils import run_bass_kernel_spmd

    if "nc" not in _cached:
        _cached["nc"] = _build()
    nc = _cached["nc"]
    in_maps = _host_inputs(np.asarray(x, np.float32),
                           np.asarray(running_amp, np.float32))
    res = run_bass_kernel_spmd(nc, in_maps, list(range(N_CORES)))
    out = np.concatenate([res.results[i]["out"] for i in range(N_CORES)], axis=0)
    return out.astype(np.float32)
